# revision 1
# baseline (speedup 1.0000x reference)
"""Trainium2 Bass kernel for nn_Decoder (LSTM over T steps + final FC).

Problem: y_hist [256, 512], LSTM(input_size=1, hidden=1024), h0/c0 [256, 1024],
output = h_T @ W_fc.T + b_fc -> [256, 1].

Sharding: data-parallel. Batch 256 -> 8 cores x 32 rows. LSTM + fc weights
replicated on every core; the time recurrence stays local per core (no
collectives).

Per-core per-step compute (batch-in-partitions orientation, column-tiled):
  gates[32, 4096] = h[32,1024] @ W_hh^T  (+ x_t * w_in + bias)
  - The 128x128 PE array runs 4 concurrent M=32 matmuls via column tiling
    (tile_position=(0, 32q)); operands bf16 (walrus rejects col-tiled
    fp32/f32r), accumulation fp32 in PSUM. Cell state c stays fp32.
  - PE column group q (PSUM partitions 32q:32q+32) computes all four gates
    for H-quarter q. PSUM PS [128, 1024]: cols [0:256)=i, [256:512)=f,
    [512:768)=g, [768:1024)=o; partition 32q+b <-> (batch b, H-col 256q+n).
    So i/f/g/o/c are all partition-aligned [128, 256] tiles (per-lane
    engines cannot cross partitions).
  - x_t*w_in + bias enters as a K=2 matmul (rows {x_t, ones} x {w_in, bias})
    with start=True; the 8 K-tiles of h @ W_hh^T accumulate on top.
  - h_new [128, 256] is re-transposed to hT (h^T, K-tile-major with column
    order HT_ORDER) by 4 PE transposes of [64,128] blocks for the next
    step's stationary operand.
  - Final FC: per-partition dot + reduce; the cross-partition 4-way add is
    done exactly via a DRAM round-trip rearrange [128,1] -> [32,4].
"""

import numpy as np
import ml_dtypes

import concourse.bass as bass
import concourse.mybir as mybir
from concourse import bacc
from concourse.tile import TileContext
from concourse.bass_utils import run_bass_kernel_spmd

B, T, H = 256, 512, 1024
NCORES = 8
BL = B // NCORES  # 32 batch rows per core
KT = H // 128  # 8 contraction tiles
F32 = mybir.dt.float32
BF16 = mybir.dt.bfloat16
NPBF16 = ml_dtypes.bfloat16

X1_CHUNK = 64  # steps per x1 SBUF prefetch chunk

# hT column layout: K-tiles stored in order [0,2,4,6,1,3,5,7] (32 cols each).
# Full [128,128] transpose of h_new cols [128c:128c+128] yields tiles
# {2q+c for q in 0..3} as its four 32-col groups.
HT_ORDER = [0, 2, 4, 6, 1, 3, 5, 7]
HT_COL = {k: 32 * HT_ORDER.index(k) for k in range(8)}

# bf16 packed blob column offsets
PKB_WT = 0
PKB_XB = PKB_WT + KT * 4096
PKB_HT0 = PKB_XB + 4096
PKB_IDN = PKB_HT0 + KT * BL
PKB_COLS = PKB_IDN + 128

# f32 packed blob column offsets
PKF_C0 = 0
PKF_WFC = PKF_C0 + 256
PKF_BFC = PKF_WFC + 256
PKF_COLS = PKF_BFC + 1


def build_nc(n_steps: int = T, repeat: int = 1, dbg_skip_elem: bool = False) -> bass.Bass:
    nc = bacc.Bacc()

    initb_d = nc.declare_dram_parameter("initb", [128, PKB_COLS], BF16, isOutput=False)
    initf_d = nc.declare_dram_parameter("initf", [128, PKF_COLS], F32, isOutput=False)
    x1_d = nc.declare_dram_parameter("x1", [2, n_steps * BL], BF16, isOutput=False)
    out_d = nc.declare_dram_parameter("out", [BL, 1], F32, isOutput=True)
    scr_d = nc.dram_tensor("scratch", [128], F32)

    with TileContext(nc) as tc:
        with (
            tc.tile_pool(name="consts", bufs=1) as consts,
            tc.tile_pool(name="state", bufs=1) as state,
            tc.tile_pool(name="x1pool", bufs=2) as x1pool,
            tc.tile_pool(name="work", bufs=2) as work,
            tc.tile_pool(name="psum", bufs=3, space="PSUM") as psum,
            tc.tile_pool(name="psumt", bufs=2, space="PSUM") as psumt,
        ):
            pkb = consts.tile([128, PKB_COLS], BF16)
            nc.sync.dma_start(out=pkb, in_=initb_d[:, :])
            pkf = consts.tile([128, PKF_COLS], F32)
            nc.sync.dma_start(out=pkf, in_=initf_d[:, :])
            wt_sb = pkb[:, PKB_WT : PKB_WT + KT * 4096]
            xb_sb = pkb[0:2, PKB_XB : PKB_XB + 4096]
            idn128 = pkb[:, PKB_IDN : PKB_IDN + 128]
            wfc_sb = pkf[:, PKF_WFC : PKF_WFC + 256]
            bfc_sb = pkf[0:BL, PKF_BFC : PKF_BFC + 1]

            # Repeat loop (timing harness: re-runs the whole recurrence).
            rep_ctx = tc.For_i(0, repeat, 1) if repeat > 1 else None
            if rep_ctx is not None:
                rep_ctx.__enter__()

            # Mutable state: copied out of the packed blobs on-chip.
            hT = state.tile([128, KT * BL], BF16)
            nc.vector.tensor_copy(hT, pkb[:, PKB_HT0 : PKB_HT0 + KT * BL])
            c_sb = state.tile([128, 256], F32)
            nc.vector.tensor_copy(c_sb, pkf[:, PKF_C0 : PKF_C0 + 256])

            x1c = None
            hnew = None

            def emit_k2(xsl, psA, psB):
                for half, pst in ((0, psA), (1, psB)):
                    for q in range(4):
                        nc.tensor.matmul(
                            pst[32 * q : 32 * q + 32, :],
                            xsl,
                            xb_sb[
                                :, 1024 * q + 512 * half : 1024 * q + 512 * half + 512
                            ],
                            start=True,
                            stop=False,
                            tile_position=(0, 32 * q),
                            skip_group_check=True,
                        )

            def emit_main_round(k, half, pst):
                lt = hT[:, HT_COL[k] : HT_COL[k] + BL]
                for q in range(4):
                    base = 4096 * k + 1024 * q + 512 * half
                    nc.tensor.matmul(
                        pst[32 * q : 32 * q + 32, :],
                        lt,
                        wt_sb[:, base : base + 512],
                        start=False,
                        stop=(k == KT - 1),
                        tile_position=(0, 32 * q),
                        skip_group_check=True,
                    )

            x1c = x1pool.tile([2, X1_CHUNK * BL], BF16, name="x1c")
            nc.sync.dma_start(
                out=x1c[:, : min(X1_CHUNK, n_steps) * BL],
                in_=x1_d[:, : min(X1_CHUNK, n_steps) * BL],
            )
            psA = psum.tile([128, 512], F32, name="psA")
            psB = psum.tile([128, 512], F32, name="psB")
            emit_k2(x1c[:, 0:BL], psA, psB)

            for t in range(n_steps):
                # Main matmuls: interleave halves for k<6 so both PSUMs fill
                # together, then finish psA early (its sigmoid overlaps the
                # trailing psB rounds).
                for k in range(KT - 2):
                    emit_main_round(k, 0, psA)
                    emit_main_round(k, 1, psB)
                for k in range(KT - 2, KT):
                    emit_main_round(k, 0, psA)
                pA, pB = psA, psB

                # next step's input-side matmul (independent of h) keeps PE
                # busy during this step's elementwise tail
                if t + 1 < n_steps:
                    u1 = (t + 1) % X1_CHUNK
                    if u1 == 0:
                        nst = min(X1_CHUNK, n_steps - (t + 1))
                        x1c = x1pool.tile([2, X1_CHUNK * BL], BF16, name="x1c")
                        nc.sync.dma_start(
                            out=x1c[:, : nst * BL],
                            in_=x1_d[:, (t + 1) * BL : (t + 1 + nst) * BL],
                        )
                    psA = psum.tile([128, 512], F32, name="psA")
                    psB = psum.tile([128, 512], F32, name="psB")
                    emit_k2(x1c[:, u1 * BL : (u1 + 1) * BL], psA, psB)

                for k in range(KT - 2, KT):
                    emit_main_round(k, 1, pB)

                if dbg_skip_elem:
                    if t != n_steps - 1:
                        psT = psumt.tile([128, KT * BL], BF16, name="psT")
                        for c in range(2):
                            nc.tensor.matmul(
                                psT[:, 128 * c : 128 * c + 128],
                                pkb[:, 128 * c : 128 * c + 128],
                                idn128,
                                is_transpose=True,
                                start=True,
                                stop=True,
                                skip_group_check=True,
                            )
                        nc.vector.tensor_copy(hT, psT)
                    continue

                # Elementwise. pA cols: [0:256)=i [256:512)=f; pB: g, o.
                # tanh(x) = 2*sigmoid(2x)-1; the 2x is pre-folded into the
                # g-gate weights, and h is kept as h/2 (the 2x folded into
                # W_hh/W_fc columns), so tanh affines collapse into
                # scalar_tensor_tensor ops and the ACT engine runs plain
                # back-to-back sigmoids.
                sif = work.tile([128, 512], F32, name="sif")
                nc.scalar.activation(
                    sif, pA, mybir.ActivationFunctionType.Sigmoid
                )
                t1 = work.tile([128, 256], F32, name="t1")
                nc.vector.tensor_mul(t1, sif[:, 256:512], c_sb)
                sgo = work.tile([128, 512], F32, name="sgo")
                nc.scalar.activation(
                    sgo, pB, mybir.ActivationFunctionType.Sigmoid
                )
                u = work.tile([128, 256], F32, name="u")
                nc.vector.scalar_tensor_tensor(
                    out=u, in0=sgo[:, 0:256], scalar=-0.5, in1=sif[:, 0:256],
                    op0=mybir.AluOpType.add, op1=mybir.AluOpType.mult,
                )
                nc.vector.scalar_tensor_tensor(
                    out=c_sb, in0=u, scalar=2.0, in1=t1,
                    op0=mybir.AluOpType.mult, op1=mybir.AluOpType.add,
                )
                sc = work.tile([128, 256], F32, name="sc")
                nc.scalar.activation(
                    sc, c_sb, mybir.ActivationFunctionType.Sigmoid, scale=2.0
                )
                hnew = work.tile([128, 256], BF16, name="hnew")
                nc.vector.scalar_tensor_tensor(
                    out=hnew, in0=sc, scalar=-0.5, in1=sgo[:, 256:512],
                    op0=mybir.AluOpType.add, op1=mybir.AluOpType.mult,
                )

                # Transpose h_new -> hT for next step: 2 full [128,128]
                # PE transposes (base partition 0 only; mixing LDW base
                # partitions between transposes wedges the device).
                if t != n_steps - 1:
                    psT = psumt.tile([128, KT * BL], BF16, name="psT")
                    for c in range(2):
                        nc.tensor.matmul(
                            psT[:, 128 * c : 128 * c + 128],
                            hnew[:, 128 * c : 128 * c + 128],
                            idn128,
                            is_transpose=True,
                            start=True,
                            stop=True,
                            skip_group_check=True,
                        )
                    nc.vector.tensor_copy(hT, psT)

            # Final FC: out[b] = sum_H h[b,H]*wfc[H] + b_fc
            if hnew is None:
                hnew = c_sb
            fcm = work.tile([128, 256], F32)
            nc.vector.tensor_mul(fcm, hnew, wfc_sb)
            fcrf = work.tile([128, 1], F32)
            nc.vector.reduce_sum(out=fcrf, in_=fcm, axis=mybir.AxisListType.X)
            # exact cross-partition 4-way add via DRAM round-trip rearrange
            nc.sync.dma_start(out=scr_d[:], in_=fcrf[:, 0])
            fcr4 = work.tile([BL, 4], F32)
            nc.sync.dma_start(
                out=fcr4, in_=scr_d.ap().rearrange("(q b) -> b q", b=BL)
            )
            fco = work.tile([BL, 1], F32)
            nc.vector.reduce_sum(out=fco, in_=fcr4, axis=mybir.AxisListType.X)
            outsb = work.tile([BL, 1], F32)
            nc.vector.tensor_scalar_add(outsb, fco, scalar1=bfc_sb)
            nc.sync.dma_start(out=out_d[:, :], in_=outsb)
            if rep_ctx is not None:
                rep_ctx.__exit__(None, None, None)

    nc.compile()
    return nc


def prep_inputs(y_hist, W_ih, W_hh, b_ih, b_hh, W_fc, b_fc, h0, c0, n_steps: int = T):
    """Build the 8 per-core input maps (host-side numpy re-layouts)."""
    f = np.float32
    W_hh = np.asarray(W_hh, f)
    w_in = np.asarray(W_ih, f)[:, 0]
    bias = (np.asarray(b_ih, f) + np.asarray(b_hh, f)).astype(f)
    W_fc = np.asarray(W_fc, f)
    b_fc = np.asarray(b_fc, f)
    y_hist = np.asarray(y_hist, f)
    h0 = np.asarray(h0, f)
    c0 = np.asarray(c0, f)

    # wt[p, 4096k + 1024q + 256gi + n] = W_hh[1024gi + 256q + n, 128k + p]
    # gi order per (k,q): 0=i 1=f 2=g 3=o. Scales folded in:
    #  - g-gate outputs pre-doubled (tanh via sigmoid(2x)),
    #  - h stored as h/2 -> all wt entries doubled, wfc doubled, ht0 halved.
    wt = np.zeros((128, KT * 4096), f)
    xb = np.zeros((2, 4096), f)
    for q in range(4):
        for gi in range(4):
            gs = 2.0 if gi == 2 else 1.0
            src = slice(1024 * gi + 256 * q, 1024 * gi + 256 * q + 256)
            for k in range(KT):
                dst = slice(
                    4096 * k + 1024 * q + 256 * gi,
                    4096 * k + 1024 * q + 256 * gi + 256,
                )
                wt[:, dst] = (2.0 * gs) * W_hh[src, 128 * k : 128 * (k + 1)].T
            xb[0, 1024 * q + 256 * gi : 1024 * q + 256 * gi + 256] = gs * w_in[src]
            xb[1, 1024 * q + 256 * gi : 1024 * q + 256 * gi + 256] = gs * bias[src]

    wfc = 2.0 * np.vstack(
        [np.tile(W_fc[0, 256 * q : 256 * (q + 1)], (32, 1)) for q in range(4)]
    ).astype(f)
    bfc = float(np.asarray(b_fc).reshape(-1)[0])
    idn128 = np.eye(128, dtype=f)

    in_maps = []
    for i in range(NCORES):
        b0 = BL * i
        ys = y_hist[b0 : b0 + BL, :n_steps]  # [32, n_steps]
        x1 = np.stack([ys.T.reshape(-1), np.ones(n_steps * BL, f)])
        h0s = 0.5 * h0[b0 : b0 + BL]
        ht0 = np.concatenate(
            [h0s[:, 128 * k : 128 * (k + 1)].T for k in HT_ORDER], axis=1
        )
        c0s = c0[b0 : b0 + BL]
        c0l = np.vstack([c0s[:, 256 * q : 256 * (q + 1)] for q in range(4)])

        pkb = np.zeros((128, PKB_COLS), NPBF16)
        pkb[:, PKB_WT : PKB_WT + KT * 4096] = wt.astype(NPBF16)
        pkb[0:2, PKB_XB : PKB_XB + 4096] = xb.astype(NPBF16)
        pkb[:, PKB_HT0 : PKB_HT0 + KT * BL] = ht0.astype(NPBF16)
        pkb[:, PKB_IDN : PKB_IDN + 128] = idn128.astype(NPBF16)

        pkf = np.zeros((128, PKF_COLS), f)
        pkf[:, PKF_C0 : PKF_C0 + 256] = c0l
        pkf[:, PKF_WFC : PKF_WFC + 256] = wfc
        pkf[0:BL, PKF_BFC] = bfc

        in_maps.append(
            {
                "initb": np.ascontiguousarray(pkb),
                "initf": np.ascontiguousarray(pkf),
                "x1": np.ascontiguousarray(x1.astype(NPBF16)),
            }
        )
    return in_maps


def run(inputs: dict, n_steps: int = T, trace: bool = False):
    nc = build_nc(n_steps)
    in_maps = prep_inputs(**inputs, n_steps=n_steps)
    res = run_bass_kernel_spmd(nc, in_maps, list(range(NCORES)), trace=trace)
    out = np.concatenate([res.results[i]["out"] for i in range(NCORES)], axis=0)
    return out, res


def kernel(**inputs) -> np.ndarray:
    out, _ = run(inputs, n_steps=T)
    return out



# revision 7
# speedup vs baseline: 1.3092x; 1.3092x over previous
"""Trainium2 Bass kernel for nn_Decoder (LSTM over T steps + final FC).

Problem: y_hist [256, 512], LSTM(input_size=1, hidden=1024), h0/c0 [256, 1024],
output = h_T @ W_fc.T + b_fc -> [256, 1].

Sharding: data-parallel. Batch 256 -> 8 cores x 32 rows. LSTM + fc weights
replicated on every core; the time recurrence stays local per core.

v2 "wavefront" schedule: the gate computation for each step is split into
two output-column phases (H-slices). The elementwise chain (sigmoid/cell
update) for phase s overlaps the PE matmuls of the other phase and of the
next step, so the tensor engine stays near-continuously busy and holds its
boosted clock (TRN2 PE p-states: ~1.2 GHz cold, 2.4 GHz after sustained
busy; idle gaps reset the boost).

Layout per core (batch rows BL=32):
  partitions p = 32q + b  <->  (H-quarter q, batch row b)
  phase s in {0,1} covers H-cols 256q + 128s + [0,128) per quarter.
  PSUM ps_s [128, 512] = gates [i|f|g|o] x 128 cols for phase s.
  Round (s,k): 4 column-tiled matmuls (tile_position (0,32q)), stationary
  hT tile k [128,32], moving wt cols [4096k+2048s+512q : +512].
  hT tile k=2q+s lives at hT[:, 128s+32q : +32]; produced per phase by one
  [128,128] PE transpose of h_s + a PSUM->SBUF copy.

Per-step PE stream (steady state): O-rounds(t) odd-k, k2(t+1), T0(t),
E-rounds(t+1) phase0 even-k, T1(t), E phase1, O(t+1)... The per-phase
chains (ACT: sigmoids; DVE: phase-0 u/c/h + copy0; GPSIMD: phase-1 +
copy1) hide under the matmuls.

Math folding (as baseline): tanh(x)=2*sigmoid(2x)-1 with the 2x folded
into the g-gate weights; h stored as h/2 (2x folded into W_hh/W_fc), so
the ACT engine runs only sigmoids and the chain is 3 DVE ops + 1 ACT op
per phase after the gate sigmoid.
"""

import numpy as np
import ml_dtypes

import concourse.bass as bass
import concourse.mybir as mybir
from concourse import bacc
from concourse.tile import TileContext
from concourse.bass_utils import run_bass_kernel_spmd

B, T, H = 256, 512, 1024
NCORES = 8
BL = B // NCORES  # 32 batch rows per core
KT = H // 128  # 8 contraction tiles
F32 = mybir.dt.float32
BF16 = mybir.dt.bfloat16
NPBF16 = ml_dtypes.bfloat16

X1_CHUNK = 64  # steps per x1 SBUF prefetch chunk

# hT col offset for contraction tile k (tile k=2q+s at 128s+32q)
def ht_col(k: int) -> int:
    return 128 * (k % 2) + 32 * (k // 2)

EVK = [0, 2, 4, 6]
ODK = [1, 3, 5, 7]

# bf16 packed blob column offsets
PKB_WT = 0
PKB_XB = PKB_WT + KT * 4096
PKB_HT0 = PKB_XB + 4096
PKB_IDN = PKB_HT0 + 256
PKB_COLS = PKB_IDN + 128

# f32 packed blob column offsets
PKF_C0 = 0
PKF_WFC = PKF_C0 + 256
PKF_BFC = PKF_WFC + 256
PKF_COLS = PKF_BFC + 1


def build_nc(n_steps: int = T) -> bass.Bass:
    nc = bacc.Bacc()

    initb_d = nc.declare_dram_parameter("initb", [128, PKB_COLS], BF16, isOutput=False)
    initf_d = nc.declare_dram_parameter("initf", [128, PKF_COLS], F32, isOutput=False)
    x1_d = nc.declare_dram_parameter("x1", [2, n_steps * BL], BF16, isOutput=False)
    out_d = nc.declare_dram_parameter("out", [BL, 1], F32, isOutput=True)
    scr_d = nc.dram_tensor("scratch", [128], F32)

    SIG = mybir.ActivationFunctionType.Sigmoid

    with TileContext(nc) as tc:
        with (
            tc.tile_pool(name="consts", bufs=1) as consts,
            tc.tile_pool(name="state", bufs=1) as state,
            tc.tile_pool(name="x1pool", bufs=2) as x1pool,
            tc.tile_pool(name="work", bufs=2) as work,
            tc.tile_pool(name="psum", bufs=3, space="PSUM") as psum,
            tc.tile_pool(name="psumt", bufs=1, space="PSUM") as psumt,
        ):
            pkb = consts.tile([128, PKB_COLS], BF16)
            nc.sync.dma_start(out=pkb, in_=initb_d[:, :])
            pkf = consts.tile([128, PKF_COLS], F32)
            nc.sync.dma_start(out=pkf, in_=initf_d[:, :])
            wt_sb = pkb[:, PKB_WT : PKB_WT + KT * 4096]
            xb_sb = pkb[0:2, PKB_XB : PKB_XB + 4096]
            idn128 = pkb[:, PKB_IDN : PKB_IDN + 128]
            wfc_sb = pkf[:, PKF_WFC : PKF_WFC + 256]
            bfc_sb = pkf[0:BL, PKF_BFC : PKF_BFC + 1]

            # Mutable state
            hT = state.tile([128, 256], BF16)
            nc.vector.tensor_copy(hT, pkb[:, PKB_HT0 : PKB_HT0 + 256])
            c_sb = state.tile([128, 256], F32)
            nc.vector.tensor_copy(c_sb, pkf[:, PKF_C0 : PKF_C0 + 256])

            def emit_k2(pst, s, xsl):
                for q in range(4):
                    nc.tensor.matmul(
                        pst[32 * q : 32 * q + 32, :],
                        xsl,
                        xb_sb[:, 2048 * s + 512 * q : 2048 * s + 512 * q + 512],
                        start=True,
                        stop=False,
                        tile_position=(0, 32 * q),
                        skip_group_check=True,
                    )

            def emit_round(pst, s, k, stop):
                lt = hT[:, ht_col(k) : ht_col(k) + 32]
                for q in range(4):
                    base = 4096 * k + 2048 * s + 512 * q
                    nc.tensor.matmul(
                        pst[32 * q : 32 * q + 32, :],
                        lt,
                        wt_sb[:, base : base + 512],
                        start=False,
                        stop=stop,
                        tile_position=(0, 32 * q),
                        skip_group_check=True,
                    )

            # --- step 0: prologue (k2 + all 16 rounds with initial hT) ---
            x1c = x1pool.tile([2, X1_CHUNK * BL], BF16, name="x1c")
            nst = min(X1_CHUNK, n_steps)
            nc.sync.dma_start(out=x1c[:, : nst * BL], in_=x1_d[:, : nst * BL])
            ps0 = psum.tile([128, 512], F32, name="ps0")
            ps1 = psum.tile([128, 512], F32, name="ps1")
            emit_k2(ps0, 0, x1c[:, 0:BL])
            emit_k2(ps1, 1, x1c[:, 0:BL])
            for s, pst in ((0, ps0), (1, ps1)):
                for k in EVK:
                    emit_round(pst, s, k, False)
            for s, pst in ((0, ps0), (1, ps1)):
                for k in ODK:
                    emit_round(pst, s, k, k == ODK[-1])

            hcat = None
            for t in range(n_steps):
                last = t == n_steps - 1
                # --- elementwise chains for step t ---
                # ACT emission order: SIGA0, SIGO0, SIGA1, SC0, SIGO1, SC1
                # DVE: U0, C0, TT1, H0, [CP0]
                # GPS: TT0, U1, C1, H1, [CP1]
                sig0 = work.tile([128, 512], F32, name="sig0")
                sig1 = work.tile([128, 512], F32, name="sig1")
                u0 = work.tile([128, 128], F32, name="u0")
                u1 = work.tile([128, 128], F32, name="u1")
                t10 = work.tile([128, 128], F32, name="t10")
                t11 = work.tile([128, 128], F32, name="t11")
                sc0 = work.tile([128, 128], F32, name="sc0")
                sc1 = work.tile([128, 128], F32, name="sc1")
                hcat = work.tile([128, 256], BF16, name="hcat")

                c0v = c_sb[:, 0:128]
                c1v = c_sb[:, 128:256]

                # Emission sequence realizes per-engine orders:
                #  ACT: SIGA0 SIGO0 SIGA1 SC0 SIGO1 SC1 CP1
                #  DVE: U0 C0 U1 H0 C1 CP0 H1
                #  GPS: TT0 TT1
                nc.scalar.activation(sig0[:, 0:384], ps0[:, 0:384], SIG)  # SIGA0
                nc.scalar.activation(sig0[:, 384:512], ps0[:, 384:512], SIG)  # SIGO0
                nc.gpsimd.tensor_mul(t10, sig0[:, 128:256], c0v)  # TT0: f*c
                nc.vector.scalar_tensor_tensor(  # U0
                    out=u0, in0=sig0[:, 256:384], scalar=-0.5, in1=sig0[:, 0:128],
                    op0=mybir.AluOpType.add, op1=mybir.AluOpType.mult,
                )
                nc.vector.scalar_tensor_tensor(  # C0
                    out=c0v, in0=u0, scalar=2.0, in1=t10,
                    op0=mybir.AluOpType.mult, op1=mybir.AluOpType.add,
                )
                nc.scalar.activation(sig1[:, 0:384], ps1[:, 0:384], SIG)  # SIGA1
                nc.gpsimd.tensor_mul(t11, sig1[:, 128:256], c1v)  # TT1 (reads old c1)
                nc.scalar.activation(sc0, c0v, SIG, scale=2.0)  # SC0
                nc.vector.scalar_tensor_tensor(  # U1
                    out=u1, in0=sig1[:, 256:384], scalar=-0.5, in1=sig1[:, 0:128],
                    op0=mybir.AluOpType.add, op1=mybir.AluOpType.mult,
                )
                nc.scalar.activation(sig1[:, 384:512], ps1[:, 384:512], SIG)  # SIGO1
                nc.vector.scalar_tensor_tensor(  # H0
                    out=hcat[:, 0:128], in0=sc0, scalar=-0.5, in1=sig0[:, 384:512],
                    op0=mybir.AluOpType.add, op1=mybir.AluOpType.mult,
                )
                nc.vector.scalar_tensor_tensor(  # C1
                    out=c1v, in0=u1, scalar=2.0, in1=t11,
                    op0=mybir.AluOpType.mult, op1=mybir.AluOpType.add,
                )
                nc.scalar.activation(sc1, c1v, SIG, scale=2.0)  # SC1

                if last:
                    nc.vector.scalar_tensor_tensor(  # H1 (no PE tail on last step)
                        out=hcat[:, 128:256], in0=sc1, scalar=-0.5,
                        in1=sig1[:, 384:512],
                        op0=mybir.AluOpType.add, op1=mybir.AluOpType.mult,
                    )
                    break

                # --- PE tail interleaved with chain tails ---
                u1x = (t + 1) % X1_CHUNK
                if u1x == 0:
                    nst = min(X1_CHUNK, n_steps - (t + 1))
                    x1c = x1pool.tile([2, X1_CHUNK * BL], BF16, name="x1c")
                    nc.sync.dma_start(
                        out=x1c[:, : nst * BL],
                        in_=x1_d[:, (t + 1) * BL : (t + 1 + nst) * BL],
                    )
                xsl = x1c[:, u1x * BL : (u1x + 1) * BL]
                psn0 = psum.tile([128, 512], F32, name="ps0")
                psn1 = psum.tile([128, 512], F32, name="ps1")
                emit_k2(psn0, 0, xsl)
                emit_k2(psn1, 1, xsl)

                # T0 + CP0 (DVE)
                psT0 = psumt.tile([128, 128], BF16, name="psT0")
                nc.tensor.matmul(
                    psT0, hcat[:, 0:128], idn128,
                    is_transpose=True, start=True, stop=True,
                    skip_group_check=True,
                )
                nc.vector.tensor_copy(hT[:, 0:128], psT0)  # CP0

                # E rounds phase 0 (even k)
                for k in EVK:
                    emit_round(psn0, 0, k, False)

                nc.vector.scalar_tensor_tensor(  # H1
                    out=hcat[:, 128:256], in0=sc1, scalar=-0.5, in1=sig1[:, 384:512],
                    op0=mybir.AluOpType.add, op1=mybir.AluOpType.mult,
                )
                # T1 + CP1 (ACT; GPSIMD can't read PSUM)
                psT1 = psumt.tile([128, 128], BF16, name="psT1")
                nc.tensor.matmul(
                    psT1, hcat[:, 128:256], idn128,
                    is_transpose=True, start=True, stop=True,
                    skip_group_check=True,
                )
                nc.scalar.copy(hT[:, 128:256], psT1)  # CP1

                # E rounds phase 1 (even k), then O rounds (odd k)
                for k in EVK:
                    emit_round(psn1, 1, k, False)
                for k in ODK:
                    emit_round(psn0, 0, k, k == ODK[-1])
                for k in ODK:
                    emit_round(psn1, 1, k, k == ODK[-1])
                ps0, ps1 = psn0, psn1

            # --- Final FC: out[b] = sum_H h[b,H]*wfc[H] + b_fc ---
            fcm = work.tile([128, 256], F32)
            nc.vector.tensor_mul(fcm, hcat, wfc_sb)
            fcrf = work.tile([128, 1], F32)
            nc.vector.reduce_sum(out=fcrf, in_=fcm, axis=mybir.AxisListType.X)
            nc.sync.dma_start(out=scr_d[:], in_=fcrf[:, 0])
            fcr4 = work.tile([BL, 4], F32)
            nc.sync.dma_start(
                out=fcr4, in_=scr_d.ap().rearrange("(q b) -> b q", b=BL)
            )
            fco = work.tile([BL, 1], F32)
            nc.vector.reduce_sum(out=fco, in_=fcr4, axis=mybir.AxisListType.X)
            outsb = work.tile([BL, 1], F32)
            nc.vector.tensor_scalar_add(outsb, fco, scalar1=bfc_sb)
            nc.sync.dma_start(out=out_d[:, :], in_=outsb)

    nc.compile()
    return nc


def prep_inputs(y_hist, W_ih, W_hh, b_ih, b_hh, W_fc, b_fc, h0, c0, n_steps: int = T):
    """Build the 8 per-core input maps (host-side numpy re-layouts)."""
    f = np.float32
    W_hh = np.asarray(W_hh, f)
    w_in = np.asarray(W_ih, f)[:, 0]
    bias = (np.asarray(b_ih, f) + np.asarray(b_hh, f)).astype(f)
    W_fc = np.asarray(W_fc, f)
    b_fc = np.asarray(b_fc, f)
    y_hist = np.asarray(y_hist, f)
    h0 = np.asarray(h0, f)
    c0 = np.asarray(c0, f)

    # wt[p, 4096k + 2048s + 512q + 128gi + j]
    #   = 2*gs * W_hh[1024gi + 256q + 128s + j, 128k + p],  gs=2 for g-gate
    wt = np.zeros((128, KT * 4096), f)
    xb = np.zeros((2, 4096), f)
    for s in range(2):
        for q in range(4):
            for gi in range(4):
                gs = 2.0 if gi == 2 else 1.0
                src = slice(
                    1024 * gi + 256 * q + 128 * s,
                    1024 * gi + 256 * q + 128 * s + 128,
                )
                for k in range(KT):
                    dst = slice(
                        4096 * k + 2048 * s + 512 * q + 128 * gi,
                        4096 * k + 2048 * s + 512 * q + 128 * gi + 128,
                    )
                    wt[:, dst] = (2.0 * gs) * W_hh[src, 128 * k : 128 * (k + 1)].T
                d2 = slice(
                    2048 * s + 512 * q + 128 * gi,
                    2048 * s + 512 * q + 128 * gi + 128,
                )
                xb[0, d2] = gs * w_in[src]
                xb[1, d2] = gs * bias[src]

    wfc = 2.0 * np.vstack(
        [np.tile(W_fc[0, 256 * q : 256 * (q + 1)], (32, 1)) for q in range(4)]
    ).astype(f)
    bfc = float(np.asarray(b_fc).reshape(-1)[0])
    idn128 = np.eye(128, dtype=f)

    in_maps = []
    for i in range(NCORES):
        b0 = BL * i
        ys = y_hist[b0 : b0 + BL, :n_steps]  # [32, n_steps]
        x1 = np.stack([ys.T.reshape(-1), np.ones(n_steps * BL, f)])
        h0s = 0.5 * h0[b0 : b0 + BL]
        # hT0[kappa, 128s+32q+b] = h0s[b, 256q+128s+kappa]
        ht0 = np.zeros((128, 256), f)
        for k in range(KT):
            q, s = k // 2, k % 2
            ht0[:, 128 * s + 32 * q : 128 * s + 32 * q + 32] = h0s[
                :, 256 * q + 128 * s : 256 * q + 128 * s + 128
            ].T
        c0s = c0[b0 : b0 + BL]
        c0l = np.vstack([c0s[:, 256 * q : 256 * (q + 1)] for q in range(4)])

        pkb = np.zeros((128, PKB_COLS), NPBF16)
        pkb[:, PKB_WT : PKB_WT + KT * 4096] = wt.astype(NPBF16)
        pkb[0:2, PKB_XB : PKB_XB + 4096] = xb.astype(NPBF16)
        pkb[:, PKB_HT0 : PKB_HT0 + 256] = ht0.astype(NPBF16)
        pkb[:, PKB_IDN : PKB_IDN + 128] = idn128.astype(NPBF16)

        pkf = np.zeros((128, PKF_COLS), f)
        pkf[:, PKF_C0 : PKF_C0 + 256] = c0l
        pkf[:, PKF_WFC : PKF_WFC + 256] = wfc
        pkf[0:BL, PKF_BFC] = bfc

        in_maps.append(
            {
                "initb": np.ascontiguousarray(pkb),
                "initf": np.ascontiguousarray(pkf),
                "x1": np.ascontiguousarray(x1.astype(NPBF16)),
            }
        )
    return in_maps


def run(inputs: dict, n_steps: int = T, trace: bool = False):
    nc = build_nc(n_steps)
    in_maps = prep_inputs(**inputs, n_steps=n_steps)
    res = run_bass_kernel_spmd(nc, in_maps, list(range(NCORES)), trace=trace)
    out = np.concatenate([res.results[i]["out"] for i in range(NCORES)], axis=0)
    return out, res


def kernel(**inputs) -> np.ndarray:
    out, _ = run(inputs, n_steps=T)
    return out
